# revision 1
# baseline (speedup 1.0000x reference)
"""BinaryTreeCRF inside algorithm on 8 Trainium2 NeuronCores.

Math per internal node p: inside[p] = em[p] + lsm_{l,r}(left[l] + right[r]
+ trans[p,l,r]).  Device state is LABEL-MAJOR and SHIFTED, one [66, *] SBUF
tile per kernel holding every level block:
    rows 0:32  = u = the level's ln-output (ACT writes it directly)
    row  32    = 1.0 (carries the runtime recentring constant -gamma via
                 row 32 of the wR weights)
    rows 33:65 = the level's emissions (host-uploaded, pre-transposed)
true score = u + em + off_t with off_t a per-level scalar; the per-level
pipeline is exactly 4 engine hops, with NO reduce/subtract/transpose/
deinterleave/add on the critical path:

    B    = wLx^T @ state[:, even cols] + wR^T @ state[:, odd]   (PE)
    outer= exp(B)                                               (ACT, bf16)
    S    = sum_c texp_c^T @ outer_c                             (PE, PSUM)
    u'   = ln(S) -> next level's state rows 0:32                (ACT, bf16)

Recentring: gamma_{t+1} = 2*max(u_t[0, :f]) + 7.5 measured at levels
1/3/5/7 (corrections hide behind the level's later work tiles) and a
constant GEXT for skipped levels (the state is centred near 0 after an
accurate correction, so the extrapolated shift is data-independent).
Gammas are bf16-quantized on device and exported (gout) so the host can
reconstruct the exact offsets off = sum_t 2^(T-t) gamma_t.  Kernel B's
gammas are host constants baked into three wR variants (no side chain).

Startup: inputs packed into one SP DMA (wR+texp+leaf half; kernel B: its
whole 15-column state too) + wLx on the Pool SWDGE queue in parallel;
dummy PE matmuls ramp the tensor-engine pstate during the DMA wait.

Sharding: core i owns the subtree over leaves [1024*i, 1024*(i+1)) and
runs 10 levels with zero communication (kernel A, SPMD x8).  The host
combines the 8 subtree roots (adds each core's offset, recentres per node
into the level-1 emissions) and kernel B (1 core) runs the top 3 levels.
"""

import numpy as np
import ml_dtypes

import concourse.bass as bass
import concourse.mybir as mybir
import concourse.tile as tile
from concourse import bass_utils

L = 32
H = 2 * L + 2   # 66: u rows 0:32, ones row 32, em rows 33:65, pad
N_LEAVES = 8192
N_CORES = 8
LPC = N_LEAVES // N_CORES
NWARM = 11

F32 = mybir.dt.float32
BF16 = mybir.dt.bfloat16
BF = ml_dtypes.bfloat16
MULT = mybir.AluOpType.mult
ADD = mybir.AluOpType.add
GEXT = 6.875   # extrapolated gamma for skip levels (state centred near 0)


def _level_sizes(n0):
    out = []
    n = n0
    while n > 1:
        n //= 2
        out.append(n)
    return out


def _plan(n0):
    sizes = _level_sizes(n0)
    plan = []
    corr = {1: [2, 3], 3: [4, 5], 5: [6, 7], 7: [8, 9, 10]} \
        if n0 == LPC else {}
    ntmap = {1: 4, 2: 4, 3: 4, 4: 2, 6: 2, 8: 2, 9: 2}
    for li, n in enumerate(sizes):
        t = li + 1
        if n0 == LPC and t in ntmap:
            ntiles = ntmap[t]
        elif n0 != LPC and t in (1, 2):
            ntiles = 2
        elif t in corr:
            ntiles = 2 if n >= 2 else 1
        else:
            ntiles = 2 if n > 128 else 1
        f = n // ntiles
        split = f >= 128
        plan.append((t, n, f, ntiles, split, corr.get(t, [])))
    return sizes, plan


def build_tree_lm(n0, num_devices):
    nc = bass.Bass("TRN2", target_bir_lowering=False, debug=False,
                   num_devices=num_devices)
    sizes, plan = _plan(n0)
    T = len(sizes)
    n_em = sum(sizes)
    isA = n0 == LPC
    n_wr = 1 if isA else T
    cwid = 128 * n_wr + 8 * L
    ncols = n0 + n_em

    offs = [0]
    for n in [n0] + sizes[:-1]:
        offs.append(offs[-1] + n)

    if isA:
        pack_d = nc.dram_tensor("pack1", [128, cwid + 512], BF16,
                                kind="ExternalInput")
        leaf1_d = nc.dram_tensor("leaf1", [H, 512], BF16,
                                 kind="ExternalInput")
        st_em_d = nc.dram_tensor("st_em", [H - L, n_em], BF16,
                                 kind="ExternalInput")
        gout_d = nc.dram_tensor("gout", [1, 12], BF16, kind="ExternalOutput")
    else:
        pack_d = nc.dram_tensor("pack1", [128, cwid + ncols], BF16,
                                kind="ExternalInput")
        gout_d = None
    wLx_d = nc.dram_tensor("wLx", [H, 1024], BF16, kind="ExternalInput")
    root_d = nc.dram_tensor("root_out", [L, 1], F32, kind="ExternalOutput")

    with tile.TileContext(nc) as tc:
        with (
            tc.tile_pool(name="consts", bufs=1) as cpool,
            tc.tile_pool(name="scores", bufs=1) as spool,
            tc.tile_pool(name="work", bufs=2) as wpool,
            tc.tile_pool(name="psum", bufs=1, space="PSUM") as ppool,
        ):
            pack_t = cpool.tile([128, pack_d.shape[1]], BF16, name="pack")
            wLx_t = cpool.tile([H, 1024], BF16, name="wLx")
            stall = (spool.tile([H, ncols], BF16, tag="stall", name="stall")
                     if isA else None)
            gout = cpool.tile([1, 12], BF16, name="gout") if isA else None
            root_f = cpool.tile([L, 1], F32, name="root_f")

            nc.sync.dma_start(pack_t, pack_d.ap())
            nc.gpsimd.dma_start(wLx_t, wLx_d.ap())
            if isA:
                nc.scalar.dma_start(stall[0:H, 512:1024], leaf1_d.ap())
                nc.scalar.dma_start(stall[L:H, n0:ncols], st_em_d.ap())
                z = cpool.tile([128, 128], BF16, name="warmz")
                nc.vector.memset(z, 0.0)
                for w in range(NWARM):
                    wt = ppool.tile([128, 128], F32, tag="bpA", bufs=1,
                                    name=f"warm{w}")
                    nc.tensor.matmul(wt, z, z, start=True, stop=True)

            def wR_ap(t):
                s = 0 if n_wr == 1 else (t - 1) * 128
                return pack_t[0:H, s:s + 128]

            texp = pack_t[:, 128 * n_wr:128 * n_wr + 8 * L]

            def child_ap(t, lo, hi, step):
                """children (level t-1 block) cols [lo, hi) with stride."""
                if isA:
                    if t == 1 and hi <= 512:
                        return pack_t[0:H, cwid + lo:cwid + hi:step]
                    return stall[:, offs[t - 1] + lo:offs[t - 1] + hi:step]
                b = cwid + offs[t - 1]
                return pack_t[0:H, b + lo:b + hi:step]

            def out_ap(t, lo, hi):
                if isA:
                    return stall[0:L, offs[t] + lo:offs[t] + hi]
                b = cwid + offs[t]
                return pack_t[0:L, b + lo:b + hi]

            pending = {}
            for (t, n, f, ntiles, split, corr) in plan:
                last = t == T
                outers, sts = [], []
                for ot in range(ntiles):
                    base = 2 * ot * f
                    outer = wpool.tile([128, 8 * f], BF16, tag="outer",
                                       bufs=3, name=f"outer{t}_{ot}")
                    outers.append(outer)
                    groups = [(0, 4, "bpA", 1), (4, 8, "bpB", 1)] if split \
                        else [(0, 8, "bpS", 2)]
                    for (c0, c1, tag, bufs) in groups:
                        bp = ppool.tile([128, (c1 - c0) * f], F32, tag=tag,
                                        bufs=bufs, name=tag)
                        for c in range(c0, c1):
                            cc = c - c0
                            nc.tensor.matmul(
                                bp[:, cc * f:(cc + 1) * f],
                                wLx_t[:, c * 128:(c + 1) * 128],
                                child_ap(t, base, base + 2 * f, 2),
                                start=True, stop=False)
                            nc.tensor.matmul(
                                bp[:, cc * f:(cc + 1) * f],
                                wR_ap(t),
                                child_ap(t, base + 1, base + 2 * f, 2),
                                start=False, stop=True)
                        nc.scalar.activation(
                            outer[:, c0 * f:c1 * f], bp,
                            mybir.ActivationFunctionType.Exp)

                if t + 1 in pending:
                    src = pending.pop(t + 1)
                    if src is None:
                        nc.vector.memset(wR_ap(t + 1)[L:L + 1, 0:128], -GEXT)
                    else:
                        nc.vector.tensor_copy(
                            wR_ap(t + 1)[L:L + 1, 0:128],
                            src.broadcast_to([1, 128]))

                for ot in range(ntiles):
                    st = ppool.tile([L, f], F32, tag="st", bufs=2, name="st")
                    sts.append(st)
                    for c in range(8):
                        nc.tensor.matmul(st,
                                         texp[:, c * L:(c + 1) * L],
                                         outers[ot][:, c * f:(c + 1) * f],
                                         start=(c == 0), stop=(c == 7))

                for ot in range(ntiles):
                    dst = root_f if last else out_ap(t, ot * f, (ot + 1) * f)
                    nc.scalar.activation(dst, sts[ot],
                                         mybir.ActivationFunctionType.Ln)
                    if ot == 0 and corr:
                        m_t = cpool.tile([1, 1], F32, name=f"m{t}")
                        nc.vector.reduce_max(m_t, out_ap(t, 0, f)[0:1, :],
                                             axis=mybir.AxisListType.X)
                        for d, tt in enumerate(corr, start=1):
                            if d == 1:
                                g = gout[0:1, tt:tt + 1]
                                nc.vector.tensor_scalar(
                                    g, m_t, -2.0, -7.5, op0=MULT, op1=ADD)
                                nc.vector.tensor_copy(
                                    wR_ap(tt)[L:L + 1, 0:128],
                                    g.broadcast_to([1, 128]))
                            else:
                                pending[tt] = None

            nc.sync.dma_start(root_d.ap(), root_f)
            if gout_d is not None:
                nc.gpsimd.dma_start(gout_d.ap(), gout)

    return nc


def _consts():
    wLx = np.zeros((H, 1024), np.float32)
    wR = np.zeros((H, 128), np.float32)
    for c in range(8):
        for q in range(128):
            wLx[4 * c + q // 32, c * 128 + q] = 1.0
            wLx[L + 1 + 4 * c + q // 32, c * 128 + q] = 1.0
    for q in range(128):
        wR[q % 32, q] = 1.0
        wR[L + 1 + q % 32, q] = 1.0
    return wLx, wR


_CACHE = {}
LAST_EXEC_NS = {"A": None, "B": None}


def _split_waits_json(raw, max_waits=1):
    import orjson

    bir = orjson.loads(raw)
    nextid = 900000
    for fn in bir["functions"]:
        for blk in fn["blocks"]:
            newinsts = []
            for ins in blk["instructions"]:
                si = ins.get("sync_info")
                w = (si or {}).get("on_wait") or []
                while len(w) > max_waits:
                    head, w = w[:max_waits], w[max_waits:]
                    newinsts.append({
                        "name": f"I-W{nextid}", "opcode": "NoOp",
                        "engine": ins["engine"], "ins": [], "outs": [],
                        "sync_info": {"on_update": [], "on_wait": head},
                        "debug": ins.get("debug", 0)})
                    nextid += 1
                if si is not None:
                    si["on_wait"] = w
                newinsts.append(ins)
            blk["instructions"] = newinsts
    return orjson.dumps(bir)


def _retarget_const_memsets(nc):
    """Move the Bass-preamble const-tile memsets (emitted before the
    all-engine barrier) from Pool to the idle DVE queue so the Pool SWDGE
    DMA generation starts ~700ns earlier; the barrier still orders them
    before their readers."""
    for blk in nc.m.functions[0].blocks:
        for ins in blk.instructions:
            if ins.opcode == "Memset" and ins.outs and \
                    "const-" in str(getattr(ins.outs[0], "memref", "")):
                ins.engine = mybir.EngineType.DVE
    return nc


def _get_nc(n0, num_devices):
    key = (n0, num_devices)
    if key not in _CACHE:
        nc = _retarget_const_memsets(build_tree_lm(n0, num_devices))
        patched = _split_waits_json(nc.to_json_bytes())
        nc.to_json_bytes = lambda: patched
        _CACHE[key] = nc
    return _CACHE[key]


def _to_bf(x):
    return np.ascontiguousarray(np.asarray(x).astype(BF))


def _bfr(x):
    return float(np.float32(x).astype(BF))


def kernel(leaf_emissions, internal_emissions, trans_matrix):
    leaf_emissions = np.asarray(leaf_emissions, np.float32)
    internal_emissions = np.asarray(internal_emissions, np.float32)
    trans_matrix = np.asarray(trans_matrix, np.float32)

    wLx, wR = _consts()
    t2 = np.exp(trans_matrix).transpose(1, 2, 0).reshape(1024, L)
    texp = np.concatenate(
        [t2[c * 128:(c + 1) * 128, :] for c in range(8)], axis=1)

    g_sizes = _level_sizes(N_LEAVES)
    g_offs = np.concatenate([[0], np.cumsum(g_sizes)])
    sub_levels = 10

    # ---- kernel A ----
    in_maps = []
    gamma1 = []
    for i in range(N_CORES):
        lT = leaf_emissions[i * LPC:(i + 1) * LPC].T          # [32, 1024]
        st_leaf = np.zeros((H, LPC), np.float32)
        st_leaf[0:L] = lT
        st_leaf[L] = 1.0
        em_parts = []
        for k in range(sub_levels):
            mk = g_sizes[k] // N_CORES
            o = g_offs[k] + i * mk
            em_parts.append(internal_emissions[o:o + mk])
        emT = np.concatenate(em_parts, 0).T                   # [32, 1023]
        st_em = np.zeros((H - L, emT.shape[1]), np.float32)
        st_em[0] = 1.0
        st_em[1:1 + L] = emT

        m0 = float(_to_bf(lT[0, 0:256]).astype(np.float32).max())
        c0 = _bfr(2 * m0 + 7.5)
        gamma1.append(c0)
        wR_i = wR.copy()
        wR_i[L, :] = -c0
        pack = np.zeros((128, 128 + 256 + 512), np.float32)
        pack[0:H, 0:128] = wR_i
        pack[:, 128:384] = texp
        pack[0:H, 384:896] = st_leaf[:, 0:512]
        in_maps.append({
            "pack1": _to_bf(pack),
            "leaf1": _to_bf(st_leaf[:, 512:1024]),
            "st_em": _to_bf(st_em),
            "wLx": _to_bf(wLx),
        })

    nc_a = _get_nc(LPC, N_CORES)
    res_a = bass_utils.run_bass_kernel_spmd(nc_a, in_maps,
                                            core_ids=list(range(N_CORES)))
    em_subroot = internal_emissions[g_offs[9]:g_offs[9] + 8]
    mids = np.zeros((8, L), np.float32)
    dev_g = {2, 4, 6, 8}
    for i in range(N_CORES):
        u = res_a.results[i]["root_out"][:, 0].astype(np.float32)
        g = res_a.results[i]["gout"][0].astype(np.float32)
        gam = [gamma1[i]] + [(-g[t] if t in dev_g else GEXT)
                             for t in range(2, 11)]
        off = 0.0
        for gt in gam:
            off = 2.0 * off + gt
        mids[i] = u + em_subroot[i] + off

    # ---- kernel B ----
    off0n = mids.max(axis=1)
    K = float(2.0 * off0n.mean())
    state0 = (mids - off0n[:, None]).T
    em_top = internal_emissions[g_offs[sub_levels]:]
    em1 = em_top[0:4].T.astype(np.float32).copy()
    em1 += (off0n[0::2] + off0n[1::2] - K)[None, :]
    emTB = np.concatenate([em1, em_top[4:7].T], axis=1)       # [32, 7]

    m0B = float(_to_bf(state0[0, :]).astype(np.float32).max())
    gB = [_bfr(2 * m0B + 7.5), GEXT, GEXT]
    wRs = []
    for gt in gB:
        w = wR.copy()
        w[L, :] = -gt
        wRs.append(w)
    cwid_b = 128 * 3 + 256
    pack_b = np.zeros((128, cwid_b + 15), np.float32)
    pack_b[0:H, 0:384] = np.concatenate(wRs, axis=1)
    pack_b[:, 384:640] = texp
    pack_b[0:L, 640:648] = state0
    pack_b[L, 640:655] = 1.0
    pack_b[L + 1:L + 1 + L, 648:655] = emTB

    nc_b = _get_nc(N_CORES, 1)
    res_b = bass_utils.run_bass_kernel_spmd(
        nc_b, [{"pack1": _to_bf(pack_b), "wLx": _to_bf(wLx)}],
        core_ids=[0])
    LAST_EXEC_NS["A"] = res_a.exec_time_ns
    LAST_EXEC_NS["B"] = res_b.exec_time_ns

    u_root = res_b.results[0]["root_out"][:, 0].astype(np.float32)
    offB = K / 2.0
    for gt in gB:
        offB = 2.0 * offB + gt
    return u_root + em_top[6] + offB

